# revision 2
# baseline (speedup 1.0000x reference)
"""Trainium2 Bass kernel for nn_Attention_87737591923407 (PVT-style spatial-
reduction attention with LoRA on q/v) — transfer-optimized v2.

Wall time of kernel() is dominated by the axon host<->device link, so v2:
  - ships x exactly once (fp16, [C, 2304] per core; 18.9MB total),
  - derives conv patches on device from the core's own x chunk (conv is
    sharded over OUTPUT rows, not input channels -> no patchT upload),
  - AllGathers the 4 per-core conv slices into the full 576-pos xs,
  - ships weights fp16 (cast to f32r on device),
  - returns the output as fp16,
  - caches device-resident inputs across calls (crc32-verified).

Sharding: 8 cores = 2 batches x 4 sequence chunks (2304 rows each).
Self-contained: only imports concourse (installed site package) + numpy/jax.
"""
import zlib
import concurrent.futures as _cf
import numpy as np

import jax
from jax.sharding import Mesh, PartitionSpec, NamedSharding
from jax.experimental.shard_map import shard_map

import concourse.bass as bass
import concourse.mybir as mybir
import concourse.tile as tile
from concourse import bacc
from concourse.bass2jax import (_bass_exec_p, install_neuronx_cc_hook,
                                partition_id_tensor)

# Problem constants (hardcoded per contract)
B, N, C = 2, 9216, 512
HEAD, SR, R = 8, 4, 32
D = C // HEAD                  # 64
NKV = (96 // SR) * (96 // SR)  # 576
SCALING = 4.0 / 32.0
EPS = 1e-5
SM_SCALE = float(D) ** -0.5    # 0.125

N_CORES = 8
NCHUNK = N // 4            # 2304 rows per core
NF = 256                   # q-rows per inner chunk
NCH = NCHUNK // NF         # 9 inner chunks
MLOC = NKV // 4            # 144 conv output positions per core
MPAD = 640                 # padded kv length (5 x 128)

F32 = mybir.dt.float32
F32R = mybir.dt.float32r
F16 = mybir.dt.float16
Exp = mybir.ActivationFunctionType.Exp
Ln = mybir.ActivationFunctionType.Ln
Copy = mybir.ActivationFunctionType.Copy
ADD = mybir.AluOpType.add
SUB = mybir.AluOpType.subtract
MULT = mybir.AluOpType.mult
BYPASS = mybir.AluOpType.bypass


def build_kernel(rep=1, bench=False):
    nc = bacc.Bacc("TRN2", target_bir_lowering=False, debug=False,
                   num_devices=N_CORES)

    def din(name, shape, dt=F16):
        if bench:
            return nc.dram_tensor(name, shape, dt, kind="Internal")
        return nc.dram_tensor(name, shape, dt, kind="ExternalInput")

    x16 = din("x16", [C, NCHUNK])        # x[b, chunk].T, fp16
    wsrT = din("wsrT", [16 * C, C])      # full conv weight (si sj ci) x co
    wqT = din("wqT", [C, C])
    wkT = din("wkT", [C, C])             # LN-gamma folded
    wvT = din("wvT", [C, C])             # LN-gamma folded
    wpT = din("wpT", [C, C])
    aqT = din("aqT", [C, R])
    bqT = din("bqT", [R, C])             # * SCALING
    avT = din("avT", [C, R])             # LN-gamma folded
    bvT = din("bvT", [R, C])             # * SCALING
    b_q = din("b_q", [1, C], F32)
    b_k = din("b_k", [1, C], F32)        # + w_k @ ln_b
    b_v = din("b_v", [1, C], F32)        # + w_v @ ln_b
    b_sr = din("b_sr", [128, 4], F32)    # b_sr chunked [co%128, co//128]
    b_p = din("b_p", [1, C], F32)
    avb = din("avb", [1, R], F32)        # A_v_eff @ ln_b

    out_d = nc.dram_tensor("out", [NCHUNK, C], F16, kind="ExternalOutput")

    def chunked(ap):
        return ap.rearrange("(o p) n -> p o n", p=128)

    with tile.TileContext(nc) as tc:
        with (
            tc.tile_pool(name="const", bufs=1) as cp,
            tc.tile_pool(name="xpool", bufs=1) as xp,
            tc.tile_pool(name="psA", bufs=1, space="PSUM") as psA,
            tc.tile_pool(name="psST", bufs=1, space="PSUM") as psST,
            tc.tile_pool(name="psAV", bufs=1, space="PSUM") as psAV,
            tc.tile_pool(name="psQ", bufs=2, space="PSUM") as psQ,
            tc.tile_pool(name="dram", bufs=1, space="DRAM") as dp,
        ):
            # ---------------- load weights (fp16 staging -> f32r) --------
            wtargets = [(wqT, [128, 4, C], True), (wkT, [128, 4, C], True),
                        (wvT, [128, 4, C], True), (wpT, [128, 4, C], True),
                        (aqT, [128, 4, R], True), (avT, [128, 4, R], True),
                        (bqT, [R, C], False), (bvT, [R, C], False)]
            wtiles = []
            with tc.tile_pool(name="wload", bufs=1) as wl:
                for i, (dram_t, shp, ch) in enumerate(wtargets):
                    ap = chunked(dram_t.ap()) if ch else dram_t.ap()
                    st = wl.tile(shp, F16, name=f"wst_{i}")
                    nc.gpsimd.dma_start(st[:], ap)
                    t = cp.tile(shp, F32R, name=f"wsb_{i}")
                    nc.vector.tensor_copy(t[:], st[:])
                    wtiles.append(t)
            (wq_sb, wk_sb, wv_sb, wp_sb,
             aq_sb, av_sb, bq_sb, bv_sb) = wtiles

            bias_q = cp.tile([1, C], F32R)
            nc.gpsimd.dma_start(bias_q[:], b_q.ap())
            bias_k = cp.tile([1, C], F32R)
            nc.gpsimd.dma_start(bias_k[:], b_k.ap())
            bias_v = cp.tile([1, C], F32R)
            nc.gpsimd.dma_start(bias_v[:], b_v.ap())
            bias_sr = cp.tile([128, 4], F32)
            nc.gpsimd.dma_start(bias_sr[:], b_sr.ap())
            bias_p = cp.tile([1, C], F32R)
            nc.gpsimd.dma_start(bias_p[:], b_p.ap())
            bias_av = cp.tile([1, R], F32R)
            nc.gpsimd.dma_start(bias_av[:], avb.ap())

            ones_f = cp.tile([1, 512], F32)
            nc.any.memset(ones_f[:], 1.0)
            ones_r = cp.tile([1, 512], F32R)
            nc.vector.tensor_copy(ones_r[:], ones_f[:])
            onesc = cp.tile([128, 1], F32)
            nc.any.memset(onesc[:], 1.0)

            # persistent fp16 copy of this core's x chunk
            x16_sb = xp.tile([128, 4, NCHUNK], F16)
            nc.gpsimd.dma_start(x16_sb[:], chunked(x16.ap()))
            # patch view: n = oi*384 + si*96 + oj*4 + sj  (oi 6, si 4, oj 24, sj 4)
            x_patch = x16_sb[:].rearrange(
                "p o (oi si oj sj) -> p si sj o oi oj",
                oi=6, si=4, oj=24, sj=4)

            for _rep in range(rep):
              with tc.tile_pool(name="mid", bufs=1) as mp:
                  xs_part = mp.tile([128, 4, MLOC], F32, tag="xspart")
                  with tc.tile_pool(name="convp", bufs=1) as vp:
                      pt_sb = vp.tile([128, 16, 4, MLOC], F16)
                      for si in range(4):
                          for sj in range(4):
                              nc.vector.tensor_copy(
                                  pt_sb[:, 4 * si + sj, :, :].rearrange(
                                      "p o (oi oj) -> p o oi oj", oi=6),
                                  x_patch[:, si, sj])

                      wsrv = chunked(wsrT.ap())  # [128, 64, C]
                      for M in range(4):
                          pc = psA.tile([128, 512], F32, tag="psa",
                                        name=f"conv_{_rep}_{M}")
                          for qt in range(4):
                              wsr_sb = vp.tile([128, 16, 128], F16,
                                               tag="wsr", bufs=2,
                                               name=f"wsr_{_rep}_{M}_{qt}")
                              nc.gpsimd.dma_start(
                                  wsr_sb[:],
                                  wsrv[:, 16 * qt:16 * qt + 16,
                                       128 * M:128 * M + 128])
                              for q in range(16):
                                  qg = 16 * qt + q
                                  nc.tensor.matmul(
                                      pc[:, :MLOC],
                                      wsr_sb[:, q, :],
                                      pt_sb[:, qg // 4, qg % 4, :],
                                      start=(qg == 0),
                                      stop=(qg == 63))
                          nc.vector.tensor_tensor(
                              xs_part[:, M, :], pc[:, :MLOC],
                              bias_sr[:, M:M + 1].broadcast_to((128, MLOC)),
                              ADD)

                  # ------------- AllGather over batch group --------------
                  cc_in = dp.tile([4, 128, MLOC], F32)
                  cc_out = dp.tile([16, 128, MLOC], F32)
                  nc.sync.dma_start(cc_in[:].rearrange("o p m -> p o m"),
                                    xs_part[:])
                  nc.gpsimd.collective_compute(
                      "AllGather", BYPASS,
                      replica_groups=[[0, 1, 2, 3], [4, 5, 6, 7]],
                      ins=[cc_in[:].opt()],
                      outs=[cc_out[:].opt()],
                  )
                  xs_g = mp.tile([128, 4, NKV], F32, tag="xsbuf", name="xs_g")
                  for g in range(4):
                      nc.sync.dma_start(
                          xs_g[:, :, MLOC * g:MLOC * g + MLOC],
                          cc_out[4 * g:4 * g + 4].rearrange(
                              "o p m -> p o m"))

                  # ---------------- LayerNorm stats ----------------
                  xs_sq = mp.tile([128, 4, NKV], F32, tag="scr", name="xs_sq")
                  nc.vector.tensor_tensor(xs_sq[:], xs_g[:], xs_g[:], MULT)
                  mu = cp.tile([1, NKV], F32, tag="t_mu", name=f"mu_{_rep}")
                  st_ps = psA.tile([1, 512], F32, tag="psa", name=f"st_sum_{_rep}")
                  for nh in range(2):
                      nsl = slice(288 * nh, 288 * nh + 288)
                      for K in range(4):
                          nc.tensor.matmul(st_ps[:, nsl if nh == 0 else slice(0, 288)],
                                           onesc[:], xs_g[:, K, nsl],
                                           start=(K == 0), stop=(K == 3))
                      nc.scalar.activation(mu[:, nsl], st_ps[:, nsl if nh == 0
                                                             else slice(0, 288)],
                                           Copy, scale=1.0 / C)
                  sq = cp.tile([1, NKV], F32, tag="t_sq", name=f"sq_{_rep}")
                  st_ps2 = psA.tile([1, 512], F32, tag="psa", name=f"st_sum2_{_rep}")
                  for nh in range(2):
                      nsl = slice(288 * nh, 288 * nh + 288)
                      for K in range(4):
                          nc.tensor.matmul(st_ps2[:, nsl if nh == 0 else slice(0, 288)],
                                           onesc[:], xs_sq[:, K, nsl],
                                           start=(K == 0), stop=(K == 3))
                      nc.scalar.activation(sq[:, nsl], st_ps2[:, nsl if nh == 0
                                                              else slice(0, 288)],
                                           Copy, scale=1.0 / C)
                  # var = sq - mu^2 ; rstd = exp(-0.5*ln(var+eps))
                  musq = cp.tile([1, NKV], F32, tag="t_musq", name=f"musq_{_rep}")
                  nc.vector.tensor_tensor(musq[:], mu[:], mu[:], MULT)
                  var = cp.tile([1, NKV], F32, tag="t_var", name=f"var_{_rep}")
                  nc.vector.tensor_tensor(var[:], sq[:], musq[:], SUB)
                  eps_t = cp.tile([1, 1], F32, tag="t_eps", name=f"eps_{_rep}")
                  nc.any.memset(eps_t[:], EPS)
                  lnv = cp.tile([1, NKV], F32, tag="t_lnv", name=f"lnv_{_rep}")
                  nc.scalar.activation(lnv[:], var[:], Ln, bias=eps_t[:])
                  rstd = cp.tile([1, NKV], F32, tag="t_rstd", name=f"rstd_{_rep}")
                  nc.scalar.activation(rstd[:], lnv[:], Exp, scale=-0.5)
                  mub = cp.tile([128, NKV], F32, tag="t_mub", name=f"mub_{_rep}")
                  nc.gpsimd.partition_broadcast(mub[:], mu[:], channels=128)
                  rstdb = cp.tile([128, NKV], F32, tag="t_rstdb", name=f"rstdb_{_rep}")
                  nc.gpsimd.partition_broadcast(rstdb[:], rstd[:], channels=128)

                  # z = (xs - mu) * rstd  (LN affine folded into weights)
                  z_sb = xp.tile([128, 4, NKV], F32R, tag="z_sb",
                                 name=f"z_sb_{_rep}")
                  z_f = mp.tile([128, 4, NKV], F32, tag="scr", name="z_f")
                  nc.vector.tensor_tensor(
                      z_f[:], xs_g[:],
                      mub[:, None, :].broadcast_to((128, 4, NKV)), SUB)
                  nc.vector.tensor_tensor(
                      z_sb[:], z_f[:],
                      rstdb[:, None, :].broadcast_to((128, 4, NKV)), MULT)

              kT_sb = xp.tile([128, 4, 10, 128], F32R, tag="kT_sb",
                              name=f"kT_sb_{_rep}")
              v_sb = xp.tile([128, 5, HEAD, D + 1], F32R, tag="v_sb",
                             name=f"v_sb_{_rep}")

              # ---------------- kT (with zero pad cols) ----------------
              zpad_f = cp.tile([128, 128], F32, tag="t_zpad", name=f"zpad_{_rep}")
              nc.any.memset(zpad_f[:], 0.0)
              nc.vector.tensor_copy(
                  kT_sb[:],
                  zpad_f[:, None, None, :].broadcast_to((128, 4, 10, 128)))
              for M in range(4):
                  for st_i, (m0, nw) in enumerate([(0, 256), (256, 256), (512, 64)]):
                      pk = psA.tile([128, 512], F32, tag="psa",
                                    name=f"k_{_rep}_{M}_{st_i}")
                      nsl = slice(m0, m0 + nw)
                      for K in range(4):
                          nc.tensor.matmul(pk[:, :nw],
                                           wk_sb[:, K, 128 * M:128 * M + 128],
                                           z_sb[:, K, nsl], start=(K == 0), stop=False)
                      nc.tensor.matmul(pk[:, :nw], bias_k[:, 128 * M:128 * M + 128],
                                       ones_r[:, :nw], start=False, stop=True)
                      b0 = 4 * st_i
                      nbl = nw // 128 if nw >= 128 else 1
                      wcl = min(nw, 128)
                      nc.scalar.copy(
                          kT_sb[0:64, M, b0:b0 + 2 * nbl:2, :wcl],
                          pk[0:64, :nw].rearrange("p (b w) -> p b w", w=wcl))
                      nc.scalar.copy(
                          kT_sb[64:128, M, b0 + 1:b0 + 2 * nbl:2, :wcl],
                          pk[64:128, :nw].rearrange("p (b w) -> p b w", w=wcl))

              # ---------------- v_sb (64 dims, then ones col at D) ----------------
              vscr = cp.tile([128, D + 1], F32, tag="t_vscr", name=f"vscr_{_rep}")
              nc.any.memset(vscr[:], 0.0)
              nc.any.memset(vscr[:, D:D + 1], 1.0)
              vzero = cp.tile([128, D + 1], F32, tag="t_vzero", name=f"vzero_{_rep}")
              nc.any.memset(vzero[:], 0.0)
              for mc in range(4):
                  nc.vector.tensor_copy(
                      v_sb[:, mc, :, :],
                      vscr[:, None, :].broadcast_to((128, HEAD, D + 1)))
              nc.vector.tensor_copy(
                  v_sb[0:64, 4, :, :],
                  vscr[0:64, None, :].broadcast_to((64, HEAD, D + 1)))
              nc.vector.tensor_copy(
                  v_sb[64:128, 4, :, :],
                  vzero[64:128, None, :].broadcast_to((64, HEAD, D + 1)))

              for mc in range(5):
                  mrows = 128 if mc < 4 else 64
                  pv = psA.tile([128, 512], F32, tag="psa", name=f"v_{_rep}_{mc}")
                  for K in range(4):
                      nc.tensor.matmul(pv[:mrows, :],
                                       z_sb[:, K, 128 * mc:128 * mc + mrows],
                                       wv_sb[:, K, :], start=(K == 0), stop=False)
                  nc.tensor.matmul(pv[:mrows, :], ones_r[:, :mrows], bias_v[:],
                                   start=False, stop=True)
                  nc.vector.tensor_copy(v_sb[:mrows, mc, :, 0:D], pv[:mrows, :])

              # ---------------- lora-v -> lv -> permuted add into v_sb ----------
              tv_sb = cp.tile([R, NKV], F32R, tag="t_tv", name=f"tv_{_rep}")
              for nh in range(2):
                  ptv = psA.tile([128, 512], F32, tag="psa", name=f"tv_{_rep}_{nh}")
                  nsl = slice(288 * nh, 288 * nh + 288)
                  for K in range(4):
                      nc.tensor.matmul(ptv[:R, :288], av_sb[:, K, :], z_sb[:, K, nsl],
                                       start=(K == 0), stop=False)
                  nc.tensor.matmul(ptv[:R, :288], bias_av[:], ones_r[:, :288],
                                   start=False, stop=True)
                  nc.scalar.copy(tv_sb[:, nsl], ptv[:R, :288])

              lv_dram = dp.tile([NKV * C], F32)
              lv_view = lv_dram[:].rearrange("(m c) -> m c", c=C)
              with tc.tile_pool(name="lvp", bufs=2) as lp:
                  for mc in range(5):
                      mrows = 128 if mc < 4 else 64
                      plv = psA.tile([128, 512], F32, tag="psa", name=f"lv_{_rep}_{mc}")
                      nc.tensor.matmul(plv[:mrows, :],
                                       tv_sb[:, 128 * mc:128 * mc + mrows],
                                       bv_sb[:], start=True, stop=True)
                      lv_sb = lp.tile([128, 512], F32, tag="lvsb")
                      nc.vector.tensor_copy(lv_sb[:mrows, :], plv[:mrows, :])
                      nc.sync.dma_start(lv_view[128 * mc:128 * mc + mrows, :],
                                        lv_sb[:mrows, :])
                  lv3 = lv_dram[:].rearrange("(h m dd) -> h m dd",
                                             h=HEAD, m=NKV, dd=D)
                  for mc in range(5):
                      mrows = 128 if mc < 4 else 64
                      zt = lp.tile([128, HEAD, D], F32, tag="zperm")
                      nc.sync.dma_start(
                          zt[:mrows, :, :],
                          lv3[:, 128 * mc:128 * mc + mrows, :].transpose([1, 0, 2]))
                      nc.vector.tensor_tensor(v_sb[:mrows, mc, :, 0:D],
                                              v_sb[:mrows, mc, :, 0:D],
                                              zt[:mrows, :, :], ADD)

              # ---------------- main attention loop ----------------
              with tc.tile_pool(name="stream", bufs=2) as sp:
                  for ncx in range(NCH):
                      nsl = slice(NF * ncx, NF * ncx + NF)

                      xT_sb = sp.tile([128, 4, NF], F32R, tag="xTc")
                      nc.vector.tensor_copy(xT_sb[:], x16_sb[:, :, nsl])

                      tq_sb = sp.tile([R, NF], F32R, tag="tq")
                      ptq = psQ.tile([128, 512], F32, tag="psq", name=f"tq_{_rep}_{ncx}")
                      for K in range(4):
                          nc.tensor.matmul(ptq[:R, :NF], aq_sb[:, K, :],
                                           xT_sb[:, K, :],
                                           start=(K == 0), stop=(K == 3))
                      nc.vector.tensor_copy(tq_sb[:], ptq[:R, :NF])

                      qT_sb = sp.tile([128, 4, NF], F32R, tag="qT")
                      for M in range(4):
                          pq = psQ.tile([128, 512], F32, tag="psq",
                                        name=f"q_{_rep}_{ncx}_{M}")
                          for K in range(4):
                              nc.tensor.matmul(pq[:, :NF],
                                               wq_sb[:, K, 128 * M:128 * M + 128],
                                               xT_sb[:, K, :],
                                               start=(K == 0), stop=False)
                          nc.tensor.matmul(pq[:, :NF], bq_sb[:, 128 * M:128 * M + 128],
                                           tq_sb[:], start=False, stop=False)
                          nc.tensor.matmul(pq[:, :NF], bias_q[:, 128 * M:128 * M + 128],
                                           ones_r[:, :NF], start=False, stop=True)
                          nc.vector.tensor_copy(qT_sb[:, M, :], pq[:, :NF])

                      outT_sb = sp.tile([128, 4, NF], F32R, tag="outT")
                      for hf in range(2):
                          av_ps = psAV.tile([D + 1, 4, NF], F32, tag="av",
                                            name=f"av_{_rep}_{ncx}_{hf}")
                          for hh in range(4):
                              h = 4 * hf + hh
                              hb = 64 * (h % 2)
                              hc = h // 2
                              st_ps_t = psST.tile([128, 5 * NF], F32, tag="st",
                                                  name=f"st_{_rep}_{ncx}_{h}")
                              for mc in range(5):
                                  nc.tensor.matmul(
                                      st_ps_t[:, NF * mc:NF * mc + NF],
                                      kT_sb[:, hc, 2 * mc + (h % 2), :],
                                      qT_sb[:, hc, :],
                                      start=True, stop=True)
                              est = sp.tile([128, 5 * NF], F32R, tag="est", bufs=3)
                              nc.scalar.activation(est[:], st_ps_t[:], Exp,
                                                   scale=SM_SCALE)
                              for mc in range(5):
                                  nc.tensor.matmul(av_ps[:, hh, :],
                                                   v_sb[:, mc, h, :],
                                                   est[:, NF * mc:NF * mc + NF],
                                                   start=(mc == 0), stop=(mc == 4))

                          srow = sp.tile([1, 4, NF], F32, tag="srow")
                          nc.vector.tensor_copy(srow[:], av_ps[D:D + 1, :, :])
                          rec_sb = sp.tile([1, 4, NF], F32, tag="rec")
                          nc.vector.reciprocal_approx_fast(rec_sb[:], srow[:])
                          recb = sp.tile([128, 4, NF], F32, tag="recb")
                          nc.gpsimd.partition_broadcast(recb[:], rec_sb[:],
                                                        channels=128)
                          nc.vector.tensor_tensor(
                              outT_sb[0:64, 2 * hf:2 * hf + 2, :],
                              av_ps[0:D, 0::2, :], recb[0:64, 0::2, :], MULT)
                          nc.vector.tensor_tensor(
                              outT_sb[64:128, 2 * hf:2 * hf + 2, :],
                              av_ps[0:D, 1::2, :], recb[64:128, 1::2, :], MULT)

                      for Mn in range(NF // 128):
                          po = psQ.tile([128, 512], F32, tag="psq",
                                        name=f"o_{_rep}_{ncx}_{Mn}")
                          for K in range(4):
                              nc.tensor.matmul(po[:],
                                               outT_sb[:, K, 128 * Mn:128 * Mn + 128],
                                               wp_sb[:, K, :],
                                               start=(K == 0), stop=False)
                          nc.tensor.matmul(po[:], ones_r[:, :128], bias_p[:],
                                           start=False, stop=True)
                          o_sb = sp.tile([128, C], F16, tag="osb")
                          nc.vector.tensor_copy(o_sb[:], po[:])
                          nc.sync.dma_start(
                              out_d.ap()[NF * ncx + 128 * Mn:
                                         NF * ncx + 128 * Mn + 128, :],
                              o_sb[:])

    nc.compile()
    return nc


def host_prep_weights(w_q, b_q, w_kv, b_kv, w_proj, b_proj, w_sr, b_sr,
                      ln_g, ln_b, lora_A_q, lora_B_q, lora_A_v, lora_B_v):
    """Shared (broadcast) weight tensors, fp16 transport."""
    f = np.float32
    h = np.float16
    w_k = w_kv[:C]
    w_v = w_kv[C:]
    w_k_eff = (w_k * ln_g[None, :]).astype(f)
    w_v_eff = (w_v * ln_g[None, :]).astype(f)
    b_k_eff = (b_kv[:C] + w_k @ ln_b).astype(f)
    b_v_eff = (b_kv[C:] + w_v @ ln_b).astype(f)
    A_v_eff = (lora_A_v * ln_g[None, :]).astype(f)
    avb = (lora_A_v @ ln_b).astype(f)
    B_q_s = (lora_B_q * SCALING).astype(f)
    B_v_s = (lora_B_v * SCALING).astype(f)

    w_flatT = np.ascontiguousarray(
        w_sr.transpose(2, 3, 1, 0).reshape(16 * C, C)).astype(h)

    return {
        "wsrT": w_flatT,
        "wqT": np.ascontiguousarray(w_q.T).astype(h),
        "wkT": np.ascontiguousarray(w_k_eff.T).astype(h),
        "wvT": np.ascontiguousarray(w_v_eff.T).astype(h),
        "wpT": np.ascontiguousarray(w_proj.T).astype(h),
        "aqT": np.ascontiguousarray(lora_A_q.T).astype(h),
        "bqT": np.ascontiguousarray(B_q_s.T).astype(h),
        "avT": np.ascontiguousarray(A_v_eff.T).astype(h),
        "bvT": np.ascontiguousarray(B_v_s.T).astype(h),
        "b_q": b_q.reshape(1, C).astype(f),
        "b_k": b_k_eff.reshape(1, C),
        "b_v": b_v_eff.reshape(1, C),
        "b_sr": np.ascontiguousarray(
            b_sr.reshape(4, 128).T).astype(f),
        "b_p": b_proj.reshape(1, C).astype(f),
        "avb": avb.reshape(1, R),
    }


def host_prep_x(x):
    """Per-core x chunks, fp16 transposed."""
    h = np.float16
    chunks = []
    for core in range(N_CORES):
        b, g = core // 4, core % 4
        xc = np.asarray(x[b][NCHUNK * g:NCHUNK * (g + 1), :], np.float32)
        chunks.append(np.ascontiguousarray(xc.T.astype(h)))
    return chunks


def host_prep(x, w_q, b_q, w_kv, b_kv, w_proj, b_proj, w_sr, b_sr,
              ln_g, ln_b, lora_A_q, lora_B_q, lora_A_v, lora_B_v):
    """Build the 8 per-core input dicts (test.py compatibility)."""
    shared = host_prep_weights(w_q, b_q, w_kv, b_kv, w_proj, b_proj,
                               w_sr, b_sr, ln_g, ln_b, lora_A_q, lora_B_q,
                               lora_A_v, lora_B_v)
    xch = host_prep_x(x)
    return [dict(shared, x16=xch[c]) for c in range(N_CORES)]


# ------------------------- SPMD runner (inlined) -------------------------

class _Runner:
    def __init__(self, nc, n_cores):
        install_neuronx_cc_hook()
        self.nc = nc
        self.n_cores = n_cores
        partition_name = (nc.partition_id_tensor.name
                          if nc.partition_id_tensor else None)
        in_names, out_names, out_avals = [], [], []
        for alloc in nc.m.functions[0].allocations:
            if not isinstance(alloc, mybir.MemoryLocationSet):
                continue
            name = alloc.memorylocations[0].name
            if alloc.kind == "ExternalInput":
                if name != partition_name:
                    in_names.append(name)
            elif alloc.kind == "ExternalOutput":
                out_names.append(name)
                out_avals.append(jax.core.ShapedArray(
                    tuple(alloc.tensor_shape), mybir.dt.np(alloc.dtype)))
        self.dbg_name = nc.dbg_addr.name if nc.dbg_addr is not None else None
        if self.dbg_name is not None:
            in_names.append(self.dbg_name)
        self.in_names = in_names
        self.out_names = out_names
        self.out_avals = out_avals
        self.n_params = len(in_names)
        all_in_names = list(in_names) + list(out_names)
        if partition_name is not None:
            all_in_names.append(partition_name)

        def _body(*args):
            operands = list(args)
            if partition_name is not None:
                operands.append(partition_id_tensor())
            return tuple(_bass_exec_p.bind(
                *operands,
                out_avals=tuple(out_avals),
                in_names=tuple(all_in_names),
                out_names=tuple(out_names),
                lowering_input_output_aliases=(),
                sim_require_finite=True,
                sim_require_nnan=True,
                nc=nc,
            ))

        devices = jax.devices()[:n_cores]
        self.mesh = Mesh(np.asarray(devices), ("core",))
        self.sharding = NamedSharding(self.mesh, PartitionSpec("core"))
        n_outs = len(out_avals)
        in_specs = (PartitionSpec("core"),) * (self.n_params + n_outs)
        out_specs = (PartitionSpec("core"),) * n_outs
        donate = tuple(range(self.n_params, self.n_params + n_outs))
        self.fn = jax.jit(
            shard_map(_body, mesh=self.mesh, in_specs=in_specs,
                      out_specs=out_specs, check_rep=False),
            donate_argnums=donate, keep_unused=True)
        self._outbufs = None

    def put_concat(self, arrs):
        """arrs: list (n_cores) of np arrays with identical shape -> one
        sharded device array."""
        return jax.device_put(np.concatenate(arrs, axis=0), self.sharding)

    def run(self, dev_in_by_name):
        if self._outbufs is None:
            self._outbufs = [jax.device_put(
                np.zeros((self.n_cores * a.shape[0], *a.shape[1:]), a.dtype),
                self.sharding) for a in self.out_avals]
        args = []
        for name in self.in_names:
            if name == self.dbg_name:
                args.append(self._dbg())
            else:
                args.append(dev_in_by_name[name])
        outs = self.fn(*args, *self._outbufs)
        self._outbufs = list(outs)
        return outs

    _dbg_cached = None

    def _dbg(self):
        if self._dbg_cached is None:
            self._dbg_cached = jax.device_put(
                np.zeros((self.n_cores, 2), np.uint32), self.sharding)
        return self._dbg_cached


_CACHE = {}


def _crc(a):
    a = np.ascontiguousarray(a)
    return (a.shape, a.dtype.str, zlib.crc32(a.view(np.uint8).reshape(-1)))


def kernel(x, w_q, b_q, w_kv, b_kv, w_proj, b_proj, w_sr, b_sr,
           ln_g, ln_b, lora_A_q, lora_B_q, lora_A_v, lora_B_v, H, W):
    assert int(H) == 96 and int(W) == 96
    x = np.asarray(x, np.float32)
    wargs = [np.asarray(a, np.float32) for a in
             [w_q, b_q, w_kv, b_kv, w_proj, b_proj, w_sr, b_sr,
              ln_g, ln_b, lora_A_q, lora_B_q, lora_A_v, lora_B_v]]

    if "rn" not in _CACHE:
        _CACHE["rn"] = _Runner(build_kernel(), N_CORES)
    rn = _CACHE["rn"]

    # device-resident weights (crc-verified cache)
    wfp = tuple(_crc(a) for a in wargs)
    if _CACHE.get("wfp") != wfp:
        shared = host_prep_weights(*wargs)
        _CACHE["wdev"] = {
            k: rn.put_concat([v] * N_CORES) for k, v in shared.items()}
        _CACHE["wfp"] = wfp

    # device-resident x (crc-verified cache)
    xfp = _crc(x)
    if _CACHE.get("xfp") != xfp:
        xch = host_prep_x(x)
        _CACHE["xdev"] = rn.put_concat(xch)
        _CACHE["xfp"] = xfp

    dev_in = dict(_CACHE["wdev"])
    dev_in["x16"] = _CACHE["xdev"]
    outs = rn.run(dev_in)

    # fetch fp16 output shards in parallel, cast to fp32
    out_global = outs[0]
    shards = list(out_global.addressable_shards)
    for s in shards:
        s.data.copy_to_host_async()
    out = np.empty((B, N, C), np.float32)

    def _fetch(s):
        row0 = s.index[0].start or 0
        core = row0 // NCHUNK
        b, g = core // 4, core % 4
        out[b, NCHUNK * g:NCHUNK * (g + 1), :] = np.asarray(s.data)

    with _cf.ThreadPoolExecutor(8) as ex:
        list(ex.map(_fetch, shards))
    return out


# revision 3
# speedup vs baseline: 1.3161x; 1.3161x over previous
"""Trainium2 Bass kernel for nn_Attention_87737591923407 (PVT-style spatial-
reduction attention with LoRA on q/v) — transfer-optimized v2.

Wall time of kernel() is dominated by the axon host<->device link, so v2:
  - ships x exactly once (fp16, [C, 2304] per core; 18.9MB total),
  - derives conv patches on device from the core's own x chunk (conv is
    sharded over OUTPUT rows, not input channels -> no patchT upload),
  - AllGathers the 4 per-core conv slices into the full 576-pos xs,
  - ships weights fp16 (cast to f32r on device),
  - returns the output as fp16,
  - caches device-resident inputs across calls (crc32-verified).

Sharding: 8 cores = 2 batches x 4 sequence chunks (2304 rows each).
Self-contained: only imports concourse (installed site package) + numpy/jax.
"""
import zlib
import concurrent.futures as _cf
import numpy as np

import jax
from jax.sharding import Mesh, PartitionSpec, NamedSharding
from jax.experimental.shard_map import shard_map

import concourse.bass as bass
import concourse.mybir as mybir
import concourse.tile as tile
from concourse import bacc
from concourse.bass2jax import (_bass_exec_p, install_neuronx_cc_hook,
                                partition_id_tensor)

# Problem constants (hardcoded per contract)
B, N, C = 2, 9216, 512
HEAD, SR, R = 8, 4, 32
D = C // HEAD                  # 64
NKV = (96 // SR) * (96 // SR)  # 576
SCALING = 4.0 / 32.0
EPS = 1e-5
SM_SCALE = float(D) ** -0.5    # 0.125

N_CORES = 8
NCHUNK = N // 4            # 2304 rows per core
NF = 256                   # q-rows per inner chunk
NCH = NCHUNK // NF         # 9 inner chunks
MLOC = NKV // 4            # 144 conv output positions per core
MPAD = 640                 # padded kv length (5 x 128)

F32 = mybir.dt.float32
F32R = mybir.dt.float32r
F16 = mybir.dt.float16
I8 = mybir.dt.int8

# fp16 weight blob layout: (name, rows, cols) in order
W_SEGS = [("wsrT", 16 * C, C), ("wqT", C, C), ("wkT", C, C),
          ("wvT", C, C), ("wpT", C, C), ("aqT", C, R), ("avT", C, R),
          ("bqT", R, C), ("bvT", R, C)]
WTOT = sum(r * c for _, r, c in W_SEGS)      # 5,308,416
WSH = WTOT // N_CORES                        # 663,552
OUTW = C + 4                                 # int8 cols + packed f32 scale
Exp = mybir.ActivationFunctionType.Exp
Ln = mybir.ActivationFunctionType.Ln
Copy = mybir.ActivationFunctionType.Copy
ADD = mybir.AluOpType.add
SUB = mybir.AluOpType.subtract
MULT = mybir.AluOpType.mult
BYPASS = mybir.AluOpType.bypass


def build_kernel(rep=1, bench=False):
    nc = bacc.Bacc("TRN2", target_bir_lowering=False, debug=False,
                   num_devices=N_CORES)

    def din(name, shape, dt=F16):
        if bench:
            return nc.dram_tensor(name, shape, dt, kind="Internal")
        return nc.dram_tensor(name, shape, dt, kind="ExternalInput")

    x16 = din("x16", [C, NCHUNK])        # x[b, chunk].T, fp16
    wsh = din("wsh", [WSH])              # this core's 1/8 of the weight blob
    b_q = din("b_q", [1, C], F32)
    b_k = din("b_k", [1, C], F32)        # + w_k @ ln_b
    b_v = din("b_v", [1, C], F32)        # + w_v @ ln_b
    b_sr = din("b_sr", [128, 4], F32)    # b_sr chunked [co%128, co//128]
    b_p = din("b_p", [1, C], F32)
    avb = din("avb", [1, R], F32)        # A_v_eff @ ln_b

    out_d = nc.dram_tensor("out", [NCHUNK, OUTW], I8, kind="ExternalOutput")

    def chunked(ap):
        return ap.rearrange("(o p) n -> p o n", p=128)

    with tile.TileContext(nc) as tc:
        with (
            tc.tile_pool(name="const", bufs=1) as cp,
            tc.tile_pool(name="xpool", bufs=1) as xp,
            tc.tile_pool(name="psA", bufs=1, space="PSUM") as psA,
            tc.tile_pool(name="psST", bufs=1, space="PSUM") as psST,
            tc.tile_pool(name="psAV", bufs=1, space="PSUM") as psAV,
            tc.tile_pool(name="psQ", bufs=2, space="PSUM") as psQ,
            tc.tile_pool(name="dram", bufs=1, space="DRAM") as dp,
        ):
            # ------- AllGather the weight blob, load fp16 -> f32r --------
            wsh_b = dp.tile([WSH], F16, name="wsh_b")
            nc.sync.dma_start(wsh_b[:], wsh.ap())
            blob = dp.tile([WTOT], F16, name="wblob")
            nc.gpsimd.collective_compute(
                "AllGather", BYPASS,
                replica_groups=[list(range(N_CORES))],
                ins=[wsh_b[:].opt()],
                outs=[blob[:].opt()],
            )
            wviews = {}
            off = 0
            for nm, r, c in W_SEGS:
                wviews[nm] = blob[off:off + r * c].rearrange(
                    "(r c) -> r c", c=c)
                off += r * c

            wtargets = [("wqT", [128, 4, C], True), ("wkT", [128, 4, C], True),
                        ("wvT", [128, 4, C], True), ("wpT", [128, 4, C], True),
                        ("aqT", [128, 4, R], True), ("avT", [128, 4, R], True),
                        ("bqT", [R, C], False), ("bvT", [R, C], False)]
            wtiles = []
            with tc.tile_pool(name="wload", bufs=1) as wl:
                for i, (nm, shp, ch) in enumerate(wtargets):
                    ap = chunked(wviews[nm]) if ch else wviews[nm]
                    st = wl.tile(shp, F16, name=f"wst_{i}")
                    nc.gpsimd.dma_start(st[:], ap)
                    t = cp.tile(shp, F32R, name=f"wsb_{i}")
                    nc.vector.tensor_copy(t[:], st[:])
                    wtiles.append(t)
            (wq_sb, wk_sb, wv_sb, wp_sb,
             aq_sb, av_sb, bq_sb, bv_sb) = wtiles

            bias_q = cp.tile([1, C], F32R)
            nc.gpsimd.dma_start(bias_q[:], b_q.ap())
            bias_k = cp.tile([1, C], F32R)
            nc.gpsimd.dma_start(bias_k[:], b_k.ap())
            bias_v = cp.tile([1, C], F32R)
            nc.gpsimd.dma_start(bias_v[:], b_v.ap())
            bias_sr = cp.tile([128, 4], F32)
            nc.gpsimd.dma_start(bias_sr[:], b_sr.ap())
            bias_p = cp.tile([1, C], F32R)
            nc.gpsimd.dma_start(bias_p[:], b_p.ap())
            bias_av = cp.tile([1, R], F32R)
            nc.gpsimd.dma_start(bias_av[:], avb.ap())

            ones_f = cp.tile([1, 512], F32)
            nc.any.memset(ones_f[:], 1.0)
            ones_r = cp.tile([1, 512], F32R)
            nc.vector.tensor_copy(ones_r[:], ones_f[:])
            onesc = cp.tile([128, 1], F32)
            nc.any.memset(onesc[:], 1.0)
            eps_q = cp.tile([128, 1], F32)
            nc.any.memset(eps_q[:], 1e-30)

            # persistent fp16 copy of this core's x chunk
            x16_sb = xp.tile([128, 4, NCHUNK], F16)
            nc.gpsimd.dma_start(x16_sb[:], chunked(x16.ap()))
            # patch view: n = oi*384 + si*96 + oj*4 + sj  (oi 6, si 4, oj 24, sj 4)
            x_patch = x16_sb[:].rearrange(
                "p o (oi si oj sj) -> p si sj o oi oj",
                oi=6, si=4, oj=24, sj=4)

            for _rep in range(rep):
              with tc.tile_pool(name="mid", bufs=1) as mp:
                  xs_part = mp.tile([128, 4, MLOC], F32, tag="xspart")
                  with tc.tile_pool(name="convp", bufs=1) as vp:
                      pt_sb = vp.tile([128, 16, 4, MLOC], F16)
                      for si in range(4):
                          for sj in range(4):
                              nc.vector.tensor_copy(
                                  pt_sb[:, 4 * si + sj, :, :].rearrange(
                                      "p o (oi oj) -> p o oi oj", oi=6),
                                  x_patch[:, si, sj])

                      wsrv = chunked(wviews["wsrT"])  # [128, 64, C]
                      for M in range(4):
                          pc = psA.tile([128, 512], F32, tag="psa",
                                        name=f"conv_{_rep}_{M}")
                          for qt in range(4):
                              wsr_sb = vp.tile([128, 16, 128], F16,
                                               tag="wsr", bufs=2,
                                               name=f"wsr_{_rep}_{M}_{qt}")
                              nc.gpsimd.dma_start(
                                  wsr_sb[:],
                                  wsrv[:, 16 * qt:16 * qt + 16,
                                       128 * M:128 * M + 128])
                              for q in range(16):
                                  qg = 16 * qt + q
                                  nc.tensor.matmul(
                                      pc[:, :MLOC],
                                      wsr_sb[:, q, :],
                                      pt_sb[:, qg // 4, qg % 4, :],
                                      start=(qg == 0),
                                      stop=(qg == 63))
                          nc.vector.tensor_tensor(
                              xs_part[:, M, :], pc[:, :MLOC],
                              bias_sr[:, M:M + 1].broadcast_to((128, MLOC)),
                              ADD)

                  # ------------- AllGather over batch group --------------
                  cc_in = dp.tile([4, 128, MLOC], F32)
                  cc_out = dp.tile([16, 128, MLOC], F32)
                  nc.sync.dma_start(cc_in[:].rearrange("o p m -> p o m"),
                                    xs_part[:])
                  nc.gpsimd.collective_compute(
                      "AllGather", BYPASS,
                      replica_groups=[[0, 1, 2, 3], [4, 5, 6, 7]],
                      ins=[cc_in[:].opt()],
                      outs=[cc_out[:].opt()],
                  )
                  xs_g = mp.tile([128, 4, NKV], F32, tag="xsbuf", name="xs_g")
                  for g in range(4):
                      nc.sync.dma_start(
                          xs_g[:, :, MLOC * g:MLOC * g + MLOC],
                          cc_out[4 * g:4 * g + 4].rearrange(
                              "o p m -> p o m"))

                  # ---------------- LayerNorm stats ----------------
                  xs_sq = mp.tile([128, 4, NKV], F32, tag="scr", name="xs_sq")
                  nc.vector.tensor_tensor(xs_sq[:], xs_g[:], xs_g[:], MULT)
                  mu = cp.tile([1, NKV], F32, tag="t_mu", name=f"mu_{_rep}")
                  st_ps = psA.tile([1, 512], F32, tag="psa", name=f"st_sum_{_rep}")
                  for nh in range(2):
                      nsl = slice(288 * nh, 288 * nh + 288)
                      for K in range(4):
                          nc.tensor.matmul(st_ps[:, nsl if nh == 0 else slice(0, 288)],
                                           onesc[:], xs_g[:, K, nsl],
                                           start=(K == 0), stop=(K == 3))
                      nc.scalar.activation(mu[:, nsl], st_ps[:, nsl if nh == 0
                                                             else slice(0, 288)],
                                           Copy, scale=1.0 / C)
                  sq = cp.tile([1, NKV], F32, tag="t_sq", name=f"sq_{_rep}")
                  st_ps2 = psA.tile([1, 512], F32, tag="psa", name=f"st_sum2_{_rep}")
                  for nh in range(2):
                      nsl = slice(288 * nh, 288 * nh + 288)
                      for K in range(4):
                          nc.tensor.matmul(st_ps2[:, nsl if nh == 0 else slice(0, 288)],
                                           onesc[:], xs_sq[:, K, nsl],
                                           start=(K == 0), stop=(K == 3))
                      nc.scalar.activation(sq[:, nsl], st_ps2[:, nsl if nh == 0
                                                              else slice(0, 288)],
                                           Copy, scale=1.0 / C)
                  # var = sq - mu^2 ; rstd = exp(-0.5*ln(var+eps))
                  musq = cp.tile([1, NKV], F32, tag="t_musq", name=f"musq_{_rep}")
                  nc.vector.tensor_tensor(musq[:], mu[:], mu[:], MULT)
                  var = cp.tile([1, NKV], F32, tag="t_var", name=f"var_{_rep}")
                  nc.vector.tensor_tensor(var[:], sq[:], musq[:], SUB)
                  eps_t = cp.tile([1, 1], F32, tag="t_eps", name=f"eps_{_rep}")
                  nc.any.memset(eps_t[:], EPS)
                  lnv = cp.tile([1, NKV], F32, tag="t_lnv", name=f"lnv_{_rep}")
                  nc.scalar.activation(lnv[:], var[:], Ln, bias=eps_t[:])
                  rstd = cp.tile([1, NKV], F32, tag="t_rstd", name=f"rstd_{_rep}")
                  nc.scalar.activation(rstd[:], lnv[:], Exp, scale=-0.5)
                  mub = cp.tile([128, NKV], F32, tag="t_mub", name=f"mub_{_rep}")
                  nc.gpsimd.partition_broadcast(mub[:], mu[:], channels=128)
                  rstdb = cp.tile([128, NKV], F32, tag="t_rstdb", name=f"rstdb_{_rep}")
                  nc.gpsimd.partition_broadcast(rstdb[:], rstd[:], channels=128)

                  # z = (xs - mu) * rstd  (LN affine folded into weights)
                  z_sb = xp.tile([128, 4, NKV], F32R, tag="z_sb",
                                 name=f"z_sb_{_rep}")
                  z_f = mp.tile([128, 4, NKV], F32, tag="scr", name="z_f")
                  nc.vector.tensor_tensor(
                      z_f[:], xs_g[:],
                      mub[:, None, :].broadcast_to((128, 4, NKV)), SUB)
                  nc.vector.tensor_tensor(
                      z_sb[:], z_f[:],
                      rstdb[:, None, :].broadcast_to((128, 4, NKV)), MULT)

              kT_sb = xp.tile([128, 4, 10, 128], F32R, tag="kT_sb",
                              name=f"kT_sb_{_rep}")
              v_sb = xp.tile([128, 5, HEAD, D + 1], F32R, tag="v_sb",
                             name=f"v_sb_{_rep}")

              # ---------------- kT (with zero pad cols) ----------------
              zpad_f = cp.tile([128, 128], F32, tag="t_zpad", name=f"zpad_{_rep}")
              nc.any.memset(zpad_f[:], 0.0)
              nc.vector.tensor_copy(
                  kT_sb[:],
                  zpad_f[:, None, None, :].broadcast_to((128, 4, 10, 128)))
              for M in range(4):
                  for st_i, (m0, nw) in enumerate([(0, 256), (256, 256), (512, 64)]):
                      pk = psA.tile([128, 512], F32, tag="psa",
                                    name=f"k_{_rep}_{M}_{st_i}")
                      nsl = slice(m0, m0 + nw)
                      for K in range(4):
                          nc.tensor.matmul(pk[:, :nw],
                                           wk_sb[:, K, 128 * M:128 * M + 128],
                                           z_sb[:, K, nsl], start=(K == 0), stop=False)
                      nc.tensor.matmul(pk[:, :nw], bias_k[:, 128 * M:128 * M + 128],
                                       ones_r[:, :nw], start=False, stop=True)
                      b0 = 4 * st_i
                      nbl = nw // 128 if nw >= 128 else 1
                      wcl = min(nw, 128)
                      nc.scalar.copy(
                          kT_sb[0:64, M, b0:b0 + 2 * nbl:2, :wcl],
                          pk[0:64, :nw].rearrange("p (b w) -> p b w", w=wcl))
                      nc.scalar.copy(
                          kT_sb[64:128, M, b0 + 1:b0 + 2 * nbl:2, :wcl],
                          pk[64:128, :nw].rearrange("p (b w) -> p b w", w=wcl))

              # ---------------- v_sb (64 dims, then ones col at D) ----------------
              vscr = cp.tile([128, D + 1], F32, tag="t_vscr", name=f"vscr_{_rep}")
              nc.any.memset(vscr[:], 0.0)
              nc.any.memset(vscr[:, D:D + 1], 1.0)
              vzero = cp.tile([128, D + 1], F32, tag="t_vzero", name=f"vzero_{_rep}")
              nc.any.memset(vzero[:], 0.0)
              for mc in range(4):
                  nc.vector.tensor_copy(
                      v_sb[:, mc, :, :],
                      vscr[:, None, :].broadcast_to((128, HEAD, D + 1)))
              nc.vector.tensor_copy(
                  v_sb[0:64, 4, :, :],
                  vscr[0:64, None, :].broadcast_to((64, HEAD, D + 1)))
              nc.vector.tensor_copy(
                  v_sb[64:128, 4, :, :],
                  vzero[64:128, None, :].broadcast_to((64, HEAD, D + 1)))

              for mc in range(5):
                  mrows = 128 if mc < 4 else 64
                  pv = psA.tile([128, 512], F32, tag="psa", name=f"v_{_rep}_{mc}")
                  for K in range(4):
                      nc.tensor.matmul(pv[:mrows, :],
                                       z_sb[:, K, 128 * mc:128 * mc + mrows],
                                       wv_sb[:, K, :], start=(K == 0), stop=False)
                  nc.tensor.matmul(pv[:mrows, :], ones_r[:, :mrows], bias_v[:],
                                   start=False, stop=True)
                  nc.vector.tensor_copy(v_sb[:mrows, mc, :, 0:D], pv[:mrows, :])

              # ---------------- lora-v -> lv -> permuted add into v_sb ----------
              tv_sb = cp.tile([R, NKV], F32R, tag="t_tv", name=f"tv_{_rep}")
              for nh in range(2):
                  ptv = psA.tile([128, 512], F32, tag="psa", name=f"tv_{_rep}_{nh}")
                  nsl = slice(288 * nh, 288 * nh + 288)
                  for K in range(4):
                      nc.tensor.matmul(ptv[:R, :288], av_sb[:, K, :], z_sb[:, K, nsl],
                                       start=(K == 0), stop=False)
                  nc.tensor.matmul(ptv[:R, :288], bias_av[:], ones_r[:, :288],
                                   start=False, stop=True)
                  nc.scalar.copy(tv_sb[:, nsl], ptv[:R, :288])

              lv_dram = dp.tile([NKV * C], F32)
              lv_view = lv_dram[:].rearrange("(m c) -> m c", c=C)
              with tc.tile_pool(name="lvp", bufs=2) as lp:
                  for mc in range(5):
                      mrows = 128 if mc < 4 else 64
                      plv = psA.tile([128, 512], F32, tag="psa", name=f"lv_{_rep}_{mc}")
                      nc.tensor.matmul(plv[:mrows, :],
                                       tv_sb[:, 128 * mc:128 * mc + mrows],
                                       bv_sb[:], start=True, stop=True)
                      lv_sb = lp.tile([128, 512], F32, tag="lvsb")
                      nc.vector.tensor_copy(lv_sb[:mrows, :], plv[:mrows, :])
                      nc.sync.dma_start(lv_view[128 * mc:128 * mc + mrows, :],
                                        lv_sb[:mrows, :])
                  lv3 = lv_dram[:].rearrange("(h m dd) -> h m dd",
                                             h=HEAD, m=NKV, dd=D)
                  for mc in range(5):
                      mrows = 128 if mc < 4 else 64
                      zt = lp.tile([128, HEAD, D], F32, tag="zperm")
                      nc.sync.dma_start(
                          zt[:mrows, :, :],
                          lv3[:, 128 * mc:128 * mc + mrows, :].transpose([1, 0, 2]))
                      nc.vector.tensor_tensor(v_sb[:mrows, mc, :, 0:D],
                                              v_sb[:mrows, mc, :, 0:D],
                                              zt[:mrows, :, :], ADD)

              # ---------------- main attention loop ----------------
              with tc.tile_pool(name="stream", bufs=2) as sp:
                  for ncx in range(NCH):
                      nsl = slice(NF * ncx, NF * ncx + NF)

                      xT_sb = sp.tile([128, 4, NF], F32R, tag="xTc")
                      nc.vector.tensor_copy(xT_sb[:], x16_sb[:, :, nsl])

                      tq_sb = sp.tile([R, NF], F32R, tag="tq")
                      ptq = psQ.tile([128, 512], F32, tag="psq", name=f"tq_{_rep}_{ncx}")
                      for K in range(4):
                          nc.tensor.matmul(ptq[:R, :NF], aq_sb[:, K, :],
                                           xT_sb[:, K, :],
                                           start=(K == 0), stop=(K == 3))
                      nc.vector.tensor_copy(tq_sb[:], ptq[:R, :NF])

                      qT_sb = sp.tile([128, 4, NF], F32R, tag="qT")
                      for M in range(4):
                          pq = psQ.tile([128, 512], F32, tag="psq",
                                        name=f"q_{_rep}_{ncx}_{M}")
                          for K in range(4):
                              nc.tensor.matmul(pq[:, :NF],
                                               wq_sb[:, K, 128 * M:128 * M + 128],
                                               xT_sb[:, K, :],
                                               start=(K == 0), stop=False)
                          nc.tensor.matmul(pq[:, :NF], bq_sb[:, 128 * M:128 * M + 128],
                                           tq_sb[:], start=False, stop=False)
                          nc.tensor.matmul(pq[:, :NF], bias_q[:, 128 * M:128 * M + 128],
                                           ones_r[:, :NF], start=False, stop=True)
                          nc.vector.tensor_copy(qT_sb[:, M, :], pq[:, :NF])

                      outT_sb = sp.tile([128, 4, NF], F32R, tag="outT")
                      for hf in range(2):
                          av_ps = psAV.tile([D + 1, 4, NF], F32, tag="av",
                                            name=f"av_{_rep}_{ncx}_{hf}")
                          for hh in range(4):
                              h = 4 * hf + hh
                              hb = 64 * (h % 2)
                              hc = h // 2
                              st_ps_t = psST.tile([128, 5 * NF], F32, tag="st",
                                                  name=f"st_{_rep}_{ncx}_{h}")
                              for mc in range(5):
                                  nc.tensor.matmul(
                                      st_ps_t[:, NF * mc:NF * mc + NF],
                                      kT_sb[:, hc, 2 * mc + (h % 2), :],
                                      qT_sb[:, hc, :],
                                      start=True, stop=True)
                              est = sp.tile([128, 5 * NF], F32R, tag="est", bufs=3)
                              nc.scalar.activation(est[:], st_ps_t[:], Exp,
                                                   scale=SM_SCALE)
                              for mc in range(5):
                                  nc.tensor.matmul(av_ps[:, hh, :],
                                                   v_sb[:, mc, h, :],
                                                   est[:, NF * mc:NF * mc + NF],
                                                   start=(mc == 0), stop=(mc == 4))

                          srow = sp.tile([1, 4, NF], F32, tag="srow")
                          nc.vector.tensor_copy(srow[:], av_ps[D:D + 1, :, :])
                          rec_sb = sp.tile([1, 4, NF], F32, tag="rec")
                          nc.vector.reciprocal_approx_fast(rec_sb[:], srow[:])
                          recb = sp.tile([128, 4, NF], F32, tag="recb")
                          nc.gpsimd.partition_broadcast(recb[:], rec_sb[:],
                                                        channels=128)
                          nc.vector.tensor_tensor(
                              outT_sb[0:64, 2 * hf:2 * hf + 2, :],
                              av_ps[0:D, 0::2, :], recb[0:64, 0::2, :], MULT)
                          nc.vector.tensor_tensor(
                              outT_sb[64:128, 2 * hf:2 * hf + 2, :],
                              av_ps[0:D, 1::2, :], recb[64:128, 1::2, :], MULT)

                      for Mn in range(NF // 128):
                          po = psQ.tile([128, 512], F32, tag="psq",
                                        name=f"o_{_rep}_{ncx}_{Mn}")
                          for K in range(4):
                              nc.tensor.matmul(po[:],
                                               outT_sb[:, K, 128 * Mn:128 * Mn + 128],
                                               wp_sb[:, K, :],
                                               start=(K == 0), stop=False)
                          nc.tensor.matmul(po[:], ones_r[:, :128], bias_p[:],
                                           start=False, stop=True)
                          # int8 quantize with per-row scale
                          rmx = sp.tile([128, 1], F32, tag="rmx")
                          nc.vector.tensor_reduce(
                              rmx[:], po[:], axis=mybir.AxisListType.X,
                              op=mybir.AluOpType.max,
                              apply_absolute_value=True)
                          rme = sp.tile([128, 1], F32, tag="rme")
                          nc.scalar.activation(rme[:], rmx[:], Copy,
                                               bias=1e-30)
                          rcp = sp.tile([128, 1], F32, tag="rcp")
                          nc.vector.reciprocal_approx_fast(rcp[:], rme[:])
                          r127 = sp.tile([128, 1], F32, tag="r127")
                          nc.scalar.activation(r127[:], rcp[:], Copy,
                                               scale=127.0)
                          sc = sp.tile([128, 1], F32, tag="sc")
                          nc.scalar.activation(sc[:], rme[:], Copy,
                                               scale=1.0 / 127.0)
                          q_sb = sp.tile([128, C], I8, tag="osb")
                          nc.vector.tensor_tensor(
                              q_sb[:], po[:],
                              r127[:].broadcast_to((128, C)), MULT)
                          row0 = NF * ncx + 128 * Mn
                          nc.sync.dma_start(
                              out_d.ap()[row0:row0 + 128, 0:C], q_sb[:])
                          nc.sync.dma_start(
                              out_d.ap()[row0:row0 + 128, C:OUTW],
                              sc[:].bitcast(I8))

    nc.compile()
    return nc


def host_prep_weights(w_q, b_q, w_kv, b_kv, w_proj, b_proj, w_sr, b_sr,
                      ln_g, ln_b, lora_A_q, lora_B_q, lora_A_v, lora_B_v):
    """fp16 weight blob (sharded 8-way for upload) + small f32 biases."""
    f = np.float32
    h = np.float16
    w_k = w_kv[:C]
    w_v = w_kv[C:]
    w_k_eff = (w_k * ln_g[None, :]).astype(f)
    w_v_eff = (w_v * ln_g[None, :]).astype(f)
    b_k_eff = (b_kv[:C] + w_k @ ln_b).astype(f)
    b_v_eff = (b_kv[C:] + w_v @ ln_b).astype(f)
    A_v_eff = (lora_A_v * ln_g[None, :]).astype(f)
    avb = (lora_A_v @ ln_b).astype(f)
    B_q_s = (lora_B_q * SCALING).astype(f)
    B_v_s = (lora_B_v * SCALING).astype(f)

    segs = {
        "wsrT": w_sr.transpose(2, 3, 1, 0).reshape(16 * C, C),
        "wqT": w_q.T, "wkT": w_k_eff.T, "wvT": w_v_eff.T,
        "wpT": w_proj.T, "aqT": lora_A_q.T, "avT": A_v_eff.T,
        "bqT": B_q_s.T, "bvT": B_v_s.T,
    }
    blob = np.empty(WTOT, h)
    off = 0
    for nm, r, c in W_SEGS:
        blob[off:off + r * c] = np.asarray(segs[nm], f).astype(h).ravel()
        off += r * c

    biases = {
        "b_q": b_q.reshape(1, C).astype(f),
        "b_k": b_k_eff.reshape(1, C),
        "b_v": b_v_eff.reshape(1, C),
        "b_sr": np.ascontiguousarray(
            b_sr.reshape(4, 128).T).astype(f),
        "b_p": b_proj.reshape(1, C).astype(f),
        "avb": avb.reshape(1, R),
    }
    return blob, biases


def host_prep_x(x):
    """Per-core x chunks, fp16 transposed."""
    h = np.float16
    chunks = []
    for core in range(N_CORES):
        b, g = core // 4, core % 4
        xc = np.asarray(x[b][NCHUNK * g:NCHUNK * (g + 1), :], np.float32)
        chunks.append(np.ascontiguousarray(xc.T.astype(h)))
    return chunks


def host_prep(x, w_q, b_q, w_kv, b_kv, w_proj, b_proj, w_sr, b_sr,
              ln_g, ln_b, lora_A_q, lora_B_q, lora_A_v, lora_B_v):
    """Build the 8 per-core input dicts (test.py compatibility)."""
    blob, biases = host_prep_weights(w_q, b_q, w_kv, b_kv, w_proj, b_proj,
                                     w_sr, b_sr, ln_g, ln_b, lora_A_q,
                                     lora_B_q, lora_A_v, lora_B_v)
    xch = host_prep_x(x)
    return [dict(biases, wsh=blob[WSH * c:WSH * (c + 1)], x16=xch[c])
            for c in range(N_CORES)]


# ------------------------- SPMD runner (inlined) -------------------------

class _Runner:
    def __init__(self, nc, n_cores):
        install_neuronx_cc_hook()
        self.nc = nc
        self.n_cores = n_cores
        partition_name = (nc.partition_id_tensor.name
                          if nc.partition_id_tensor else None)
        in_names, out_names, out_avals = [], [], []
        for alloc in nc.m.functions[0].allocations:
            if not isinstance(alloc, mybir.MemoryLocationSet):
                continue
            name = alloc.memorylocations[0].name
            if alloc.kind == "ExternalInput":
                if name != partition_name:
                    in_names.append(name)
            elif alloc.kind == "ExternalOutput":
                out_names.append(name)
                out_avals.append(jax.core.ShapedArray(
                    tuple(alloc.tensor_shape), mybir.dt.np(alloc.dtype)))
        self.dbg_name = nc.dbg_addr.name if nc.dbg_addr is not None else None
        if self.dbg_name is not None:
            in_names.append(self.dbg_name)
        self.in_names = in_names
        self.out_names = out_names
        self.out_avals = out_avals
        self.n_params = len(in_names)
        all_in_names = list(in_names) + list(out_names)
        if partition_name is not None:
            all_in_names.append(partition_name)

        def _body(*args):
            operands = list(args)
            if partition_name is not None:
                operands.append(partition_id_tensor())
            return tuple(_bass_exec_p.bind(
                *operands,
                out_avals=tuple(out_avals),
                in_names=tuple(all_in_names),
                out_names=tuple(out_names),
                lowering_input_output_aliases=(),
                sim_require_finite=True,
                sim_require_nnan=True,
                nc=nc,
            ))

        devices = jax.devices()[:n_cores]
        self.mesh = Mesh(np.asarray(devices), ("core",))
        self.sharding = NamedSharding(self.mesh, PartitionSpec("core"))
        n_outs = len(out_avals)
        in_specs = (PartitionSpec("core"),) * (self.n_params + n_outs)
        out_specs = (PartitionSpec("core"),) * n_outs
        donate = tuple(range(self.n_params, self.n_params + n_outs))
        self.fn = jax.jit(
            shard_map(_body, mesh=self.mesh, in_specs=in_specs,
                      out_specs=out_specs, check_rep=False),
            donate_argnums=donate, keep_unused=True)
        self._outbufs = None

    def put_concat(self, arrs):
        """arrs: list (n_cores) of np arrays with identical shape -> one
        sharded device array."""
        return jax.device_put(np.concatenate(arrs, axis=0), self.sharding)

    def run(self, dev_in_by_name):
        if self._outbufs is None:
            self._outbufs = [jax.device_put(
                np.zeros((self.n_cores * a.shape[0], *a.shape[1:]), a.dtype),
                self.sharding) for a in self.out_avals]
        args = []
        for name in self.in_names:
            if name == self.dbg_name:
                args.append(self._dbg())
            else:
                args.append(dev_in_by_name[name])
        outs = self.fn(*args, *self._outbufs)
        self._outbufs = list(outs)
        return outs  # async: caller's fetch synchronizes

    _dbg_cached = None

    def _dbg(self):
        if self._dbg_cached is None:
            self._dbg_cached = jax.device_put(
                np.zeros((self.n_cores, 2), np.uint32), self.sharding)
        return self._dbg_cached


_CACHE = {}
_POOL = _cf.ThreadPoolExecutor(8)


def _crc(a, pool=None):
    a = np.ascontiguousarray(a)
    flat = a.view(np.uint8).reshape(-1)
    if pool is not None and flat.nbytes > 1 << 22:
        k = 8
        step = (len(flat) + k - 1) // k
        parts = [flat[i * step:(i + 1) * step] for i in range(k)]
        crcs = tuple(pool.map(zlib.crc32, parts))
        return (a.shape, a.dtype.str, crcs)
    return (a.shape, a.dtype.str, zlib.crc32(flat))


def kernel(x, w_q, b_q, w_kv, b_kv, w_proj, b_proj, w_sr, b_sr,
           ln_g, ln_b, lora_A_q, lora_B_q, lora_A_v, lora_B_v, H, W):
    assert int(H) == 96 and int(W) == 96
    x = np.asarray(x, np.float32)
    wargs = [np.asarray(a, np.float32) for a in
             [w_q, b_q, w_kv, b_kv, w_proj, b_proj, w_sr, b_sr,
              ln_g, ln_b, lora_A_q, lora_B_q, lora_A_v, lora_B_v]]

    if "rn" not in _CACHE:
        _CACHE["rn"] = _Runner(build_kernel(), N_CORES)
    rn = _CACHE["rn"]

    # device-resident weights (crc-verified cache)
    wfp = tuple(_crc(a, _POOL) for a in wargs)
    if _CACHE.get("wfp") != wfp:
        blob, biases = host_prep_weights(*wargs)
        wdev = {k: rn.put_concat([v] * N_CORES) for k, v in biases.items()}
        wdev["wsh"] = rn.put_concat(
            [blob[WSH * c:WSH * (c + 1)] for c in range(N_CORES)])
        _CACHE["wdev"] = wdev
        _CACHE["wfp"] = wfp

    # device-resident x (crc-verified cache)
    xfp = _crc(x, _POOL)
    if _CACHE.get("xfp") != xfp:
        xch = host_prep_x(x)
        _CACHE["xdev"] = rn.put_concat(xch)
        _CACHE["xfp"] = xfp

    dev_in = dict(_CACHE["wdev"])
    dev_in["x16"] = _CACHE["xdev"]
    outs = rn.run(dev_in)

    # fetch int8 output shards in parallel, dequantize to fp32
    out_global = outs[0]
    shards = list(out_global.addressable_shards)
    for s in shards:
        s.data.copy_to_host_async()
    out = np.empty((B, N, C), np.float32)

    def _fetch(s):
        row0 = s.index[0].start or 0
        core = row0 // NCHUNK
        b, g = core // 4, core % 4
        raw = np.asarray(s.data)                      # [NCHUNK, 516] int8
        scale = raw[:, C:OUTW].copy().view(np.float32)  # [NCHUNK, 1]
        out[b, NCHUNK * g:NCHUNK * (g + 1), :] = (
            raw[:, :C].astype(np.float32) * scale)

    list(_POOL.map(_fetch, shards))
    return out
